# revision 25
# baseline (speedup 1.0000x reference)
"""Trainium2 Bass kernel for FlattenIntraCycleMoELayer (top-2 MoE + general path).

Strategy (v3):
  - Data-parallel over B (8 batteries per core); gen_W folded into each
    expert on host (gates sum to 1): out = x @ (g1*A_e1 + g2*A_e2),
    A_e = gen_W + expert_W[e], bias folded via an appended ones-row.
  - Gating layer-1 d_ff-sharded; inputs loaded in interleaved chunks so
    L1 matmuls start ~3us in.  gelu shortened to a 5-op chain reading
    L1 PSUM directly; layer-2 runs on a zero-padded 128-row hT so the
    partial-logits eviction covers all 128 partitions (no memset) and
    adds gate_b2/8 in the same op (the 8-core sum then includes b2 once).
  - Cross-core logits exchange: the 7 remote_dma_broadcast descriptor
    writes are emitted at the TOP of the gpsimd stream (descriptor
    generation does not read data; both HW and the interp read the
    source at TRIGGER time), so only the cheap trigger waits for the
    partial-logits eviction.  The gang-launch dummy AllReduce comes
    after the trigger, where its ncfw barrier blocks only an idle gpsimd.
  - Post-exchange: 3-op tree sum -> one select matmul to my 8 batteries
    -> top-2 via g2 = sigmoid(l2 - l1), g1 = 1 - g2 (equivalent to the
    reference's masked-softmax renorm up to the 1e-9 eps) -> one 16-col
    broadcast matmul for g1/g2 across partitions.
  - Combine is 2 fused ops per piece: t2 = g2*A_e2 (ACT scale-copy),
    wb = (g1*A_e1) + t2 (DVE scalar_tensor_tensor).
  - DMA: A streamed in per-k-tile chunks on the scalar ring (issued at
    the top of the ACT stream), gating chunks + x batteries + outputs
    on the sync ring.
  - Main loop: kt-major matmuls per battery, combine one battery ahead,
    PSUM double-buffered, evictions batched 2 m-tiles per op (DVE/ACT).

Host-side prep only reshapes/pads/casts/re-parametrizes weights;
all model math runs on device.
"""

import numpy as np
import ml_dtypes


def _ensure_import_path():
    try:
        import concourse  # noqa: F401
    except ImportError:
        import sys
        for p in ("/opt/trn_rl_repo", "/root/.axon_site/_ro/trn_rl_repo"):
            if p not in sys.path:
                sys.path.insert(0, p)
        import concourse  # noqa: F401


_ensure_import_path()

import concourse.bass as bass  # noqa: E402
import concourse.tile as tile  # noqa: E402
from concourse import mybir  # noqa: E402
from concourse.bass import ds, ts  # noqa: E402
from concourse.alu_op_type import AluOpType  # noqa: E402
from concourse.tile import add_dep_helper  # noqa: E402

BF16 = mybir.dt.bfloat16
F32 = mybir.dt.float32
F16 = mybir.dt.float16
U32 = mybir.dt.uint32

# Problem shape constants (hardcoded per contest rules).
B, L, C, F = 64, 512, 3, 300
CF = C * F              # 900
K = CF + 1              # 901 contraction rows (data + ones row for bias)
KT = 8                  # k-tiles: 7 full + 1 remainder
KREM = K - 7 * 128      # 5 rows in the last k-tile
D = 512                 # d_model
E = 8                   # experts
NCORES = 8
BPC = B // NCORES       # 8 batteries per core
DLLM = 4096
GK = 4224               # padded gating contraction = 33*128
GKT = GK // 128         # 33
DFF = 2048
DFFC = DFF // NCORES    # 256 per-core d_ff chunk
MT = L // 128           # 4 m-tiles per battery
GCHUNKS = [(0, 9), (9, 21), (21, 33)]   # gating k-tile DMA chunks
NWARM = 10              # PE warm-up junk matmuls after the exchange


def build_program(nc):
    from contextlib import ExitStack

    xmain = nc.dram_tensor("xmain", [BPC, 128, 7, L], BF16, kind="ExternalInput")
    xrem = nc.dram_tensor("xrem", [BPC, KREM, L], BF16, kind="ExternalInput")
    amain = nc.dram_tensor("amain", [128, 7, E, D], BF16, kind="ExternalInput")
    arem = nc.dram_tensor("arem", [KREM, E, D], BF16, kind="ExternalInput")
    gintp = nc.dram_tensor("gintp", [128, GKT * B], F16, kind="ExternalInput")
    w1d = nc.dram_tensor("w1", [128, GKT * DFFC], F16, kind="ExternalInput")
    w2p = nc.dram_tensor("w2p", [128, 2 * E], F32, kind="ExternalInput")
    b2d = nc.dram_tensor("b2rep", [1, E], F32, kind="ExternalInput")
    seld = nc.dram_tensor("selt", [B, BPC], F32, kind="ExternalInput")
    id64d = nc.dram_tensor("id64", [B, B], F32, kind="ExternalInput")
    outd = nc.dram_tensor("out", [BPC, 128, MT, D], BF16, kind="ExternalOutput")

    with tile.TileContext(nc) as tc, ExitStack() as ctx:
        singles = ctx.enter_context(tc.tile_pool(name="singles", bufs=1))
        gpool = ctx.enter_context(tc.tile_pool(name="gate", bufs=1))
        dpool = ctx.enter_context(tc.tile_pool(name="dram", bufs=1, space="DRAM"))

        w1_ctx = ExitStack()
        w1pool = w1_ctx.enter_context(tc.tile_pool(name="w1s", bufs=1))
        gps_ctx = ExitStack()
        gps = gps_ctx.enter_context(tc.tile_pool(name="gpsum", bufs=1, space="PSUM"))

        # ---------- DMAs -----------------------------------------------
        # sync ring: gating input chunks + small tensors + battery-0 x;
        # scalar ring: w1 chunks, then the early A k-tiles.  Gating data
        # leads both rings so layer 1 can start ~13us in.
        ginT_sb = gpool.tile([128, GKT, B], F16)
        w1_sb = w1pool.tile([128, GKT, DFFC], F16)
        gin_ap = gintp.ap().rearrange("p (k b) -> p k b", k=GKT)
        w1_ap = w1d.ap().rearrange("p (k f) -> p k f", k=GKT)
        for (lo, hi) in GCHUNKS:
            nc.sync.dma_start(out=ginT_sb[:, lo:hi, :], in_=gin_ap[:, lo:hi, :])
            nc.scalar.dma_start(out=w1_sb[:, lo:hi, :], in_=w1_ap[:, lo:hi, :])
        w2_sb = gpool.tile([128, 2, E], F32)
        nc.sync.dma_start(out=w2_sb.rearrange("p j e -> p (j e)"), in_=w2p.ap())
        b2_sb = gpool.tile([1, E], F32)
        nc.sync.dma_start(out=b2_sb, in_=b2d.ap())
        sel_sb = gpool.tile([B, BPC], F32)
        nc.sync.dma_start(out=sel_sb, in_=seld.ap())
        id64 = gpool.tile([B, B], F32)
        nc.sync.dma_start(out=id64, in_=id64d.ap())

        # scalar ring: fused expert weights A, one chunk per k-tile so the
        # combine of piece kt only waits for chunk kt.  SBUF layout is
        # e-major (combine slices [e, kt-range, :] contiguously); the DMA
        # reads the contiguous kt-major host chunk and scatters per-e.
        # ONLY k-tiles 0:AEARLY are loaded before the exchange: everything
        # queued at trigger time delays the remote sends behind it on the
        # shared SDMA engines, so the bulk (A tail + x batteries 1:8) is
        # deferred until the exchange sum (su1) has run.
        AEARLY = 5
        A_sb = singles.tile([128, E, KT, D], BF16)
        am_ap = amain.ap()
        for kt in range(AEARLY):
            nc.scalar.dma_start(
                out=A_sb[:, :, kt, :],
                in_=am_ap[:, kt, :, :],
            )
        # zero k-tile 7 (rows KREM:128 are read by the combine); the KREM
        # data rows land over it in the deferred section.
        nc.vector.memset(A_sb[:, :, 7, :], 0.0)

        # sync ring: battery 0's x only; the rest is deferred.
        xmain_ap = xmain.ap()
        xrem_ap = xrem.ap()
        xb_tiles = [None] * BPC
        for b in range(BPC):
            xb = singles.tile([128, KT, L], BF16, tag=f"xb{b}")
            xb_tiles[b] = xb

        def load_xb(b):
            first = nc.sync.dma_start(
                out=xb_tiles[b][:, 0:7, :].rearrange("p k l -> p (k l)"),
                in_=xmain_ap[b].rearrange("p k l -> p (k l)"),
            )
            nc.sync.dma_start(out=xb_tiles[b][0:KREM, 7, :], in_=xrem_ap[b])
            return first

        load_xb(0)

        # ---------- DVE constants / zero-pads (no deps, ~1us) ----------
        hT_sb = gpool.tile([128, 2, 128], F32)
        nc.vector.memset(hT_sb, 0.0)        # cols B:128 stay zero for L2
        ones_sb = gpool.tile([B, 128], F32)
        nc.vector.memset(ones_sb, 1.0)
        jt = gpool.tile([128, DFFC], F16, tag="junk")
        nc.vector.memset(jt, 0.0)

        # ---------- gating layer 1 (PE, chunk-ordered) -----------------
        # Pre-warm the PE while the first gating chunk is in flight: the
        # HAM clock-gate needs ~4us of sustained activity to unlock full
        # speed, and gating layer 1 is on the exchange-send critical path.
        psum_h = gps.tile([B, DFFC], F32, bufs=1)
        for j in range(36):
            nc.tensor.matmul(out=psum_h, lhsT=jt[:, 0:B], rhs=jt,
                             start=True, stop=True)
        for kt in range(GKT):
            nc.tensor.matmul(
                out=psum_h, lhsT=ginT_sb[:, kt, :], rhs=w1_sb[:, kt, :],
                start=(kt == 0), stop=(kt == GKT - 1),
            )

        # gelu (tanh approx):
        #   h = (0.5*x) * (1 + tanh(0.79788456*(x + 0.044715*x^3)))
        g_x = gpool.tile([B, DFFC], F32)
        nc.vector.tensor_copy(out=g_x, in_=psum_h)
        g_x2 = gpool.tile([B, DFFC], F32)
        nc.vector.tensor_tensor(out=g_x2, in0=g_x, in1=g_x, op=AluOpType.mult)
        g_xh = gpool.tile([B, DFFC], F32)
        nc.vector.tensor_scalar_mul(g_xh, g_x, 0.5)
        g_p = gpool.tile([B, DFFC], F32)
        nc.vector.tensor_scalar(g_p, g_x2, 0.044715, 1.0,
                                AluOpType.mult, AluOpType.add)
        g_u = gpool.tile([B, DFFC], F32)
        nc.vector.tensor_tensor(out=g_u, in0=g_x, in1=g_p, op=AluOpType.mult)
        g_t = gpool.tile([B, DFFC], F32)
        nc.scalar.activation(out=g_t, in_=g_u,
                             func=mybir.ActivationFunctionType.Tanh,
                             scale=0.7978845608028654)
        h_sb = gpool.tile([B, DFFC], F32)
        nc.vector.scalar_tensor_tensor(out=h_sb, in0=g_t, scalar=1.0, in1=g_xh,
                                       op0=AluOpType.add, op1=AluOpType.mult)

        # transpose h -> hT [128, 2, 0:B] (cols B:128 remain zero)
        for j in range(2):
            pst = gps.tile([128, B], F32, bufs=2, tag="pst")
            nc.tensor.transpose(
                out=pst, in_=h_sb[:, j * 128:(j + 1) * 128], identity=id64
            )
            nc.vector.tensor_copy(out=hT_sb[:, j, 0:B], in_=pst)

        # layer 2: partial logits on all 128 partitions; a third K=1
        # accumulation matmul (ones-row x b2/8-row) folds gate_b2/8 in,
        # so the 8-core sum includes b2 exactly once.
        psum_l = gps.tile([128, E], F32, bufs=2, tag="pst")
        for j in range(2):
            nc.tensor.matmul(out=psum_l, lhsT=hT_sb[:, j, :], rhs=w2_sb[:, j, :],
                             start=(j == 0), stop=False)
        nc.tensor.matmul(out=psum_l, lhsT=ones_sb[0:1, :], rhs=b2_sb,
                         start=False, stop=True)
        lgt_sb = gpool.tile([128, E], F32)
        nc.vector.tensor_copy(out=lgt_sb, in_=psum_l)

        # ---------- cross-core logits sum: real ncfw AllReduce ---------
        # The remote-DMA exchange path moves the 4KB payload as ~1000
        # 4-byte SWDGE packets (~25us serial per engine lane); the ncfw
        # AllReduce has a similar fixed latency but removes the Q7
        # descriptor-generation dispatch (~10us), the semaphore reset
        # epilogue, and all post-scheduling sem injection.  It also
        # registers the NEFF with the collectives runtime (gang launch).
        cc_in = dpool.tile([128, E], F32)
        nc.sync.dma_start(out=cc_in, in_=lgt_sb)
        cc_out = dpool.tile([128, E], F32, addr_space="Shared")
        nc.gpsimd.collective_compute(
            "AllReduce", AluOpType.add,
            replica_groups=[list(range(NCORES))],
            ins=[cc_in], outs=[cc_out],
        )
        logits_all = gpool.tile([B, E], F32)
        lgt_dma = nc.gpsimd.dma_start(out=logits_all, in_=cc_out[0:B, :])

        # PE warm-up: junk matmuls from the allreduce wait onward so the
        # HAM clock-gate is ramped when the fused matmuls begin.
        for j in range(NWARM):
            jmm = nc.tensor.matmul(
                out=psum_h, lhsT=ginT_sb[:, j, :], rhs=w1_sb[:, j, :],
                start=True, stop=True,
            )
            if j == 0:
                add_dep_helper(jmm.ins, lgt_dma.ins, sync=True,
                               reason="warm-up matmuls start at allreduce completion")

        # select my 8 batteries (one-hot matmul); b2 already included
        psum_sel = gps.tile([BPC, E], F32, bufs=2, tag="pst")
        nc.tensor.matmul(out=psum_sel, lhsT=sel_sb, rhs=logits_all,
                         start=True, stop=True)
        logits_my = gpool.tile([BPC, E], F32)
        nc.vector.tensor_copy(out=logits_my, in_=psum_sel)

        # top-2 gates: g2 = sigmoid(l2 - l1), g1 = 1 - g2
        sorted8 = gpool.tile([BPC, E], F32)
        sidx = gpool.tile([BPC, E], U32)
        nc.vector.max(out=sorted8, in_=logits_my)
        nc.vector.max_index(out=sidx, in_max=sorted8, in_values=logits_my)
        diff = gpool.tile([BPC, 1], F32)
        nc.vector.tensor_tensor(out=diff, in0=sorted8[:, 1:2],
                                in1=sorted8[:, 0:1], op=AluOpType.subtract)
        g2c = gpool.tile([BPC, 1], F32)
        nc.scalar.activation(out=g2c, in_=diff,
                             func=mybir.ActivationFunctionType.Sigmoid,
                             scale=1.0)
        g1c = gpool.tile([BPC, 1], F32)
        nc.vector.tensor_scalar(g1c, g2c, -1.0, 1.0,
                                AluOpType.mult, AluOpType.add)

        # broadcast g1/g2 of each battery to all 128 partitions with one
        # matmul: rhs = [diag(g1) | diag(g2)] (8 x 16), lhsT = ones (8 x 128).
        rhs8 = gpool.tile([BPC, 2, BPC], F32)
        nc.vector.tensor_scalar_mul(rhs8[:, 0, :], id64[0:BPC, 0:BPC], g1c)
        nc.scalar.activation(out=rhs8[:, 1, :], in_=id64[0:BPC, 0:BPC],
                             func=mybir.ActivationFunctionType.Copy,
                             scale=g2c)
        psum_bc = gps.tile([128, 2 * BPC], F32, bufs=2, tag="pbc")
        nc.tensor.matmul(out=psum_bc, lhsT=ones_sb[0:BPC, :],
                         rhs=rhs8.rearrange("p s b -> p (s b)"),
                         start=True, stop=True)
        bcA = gpool.tile([128, 2, BPC], F32)
        nc.vector.tensor_copy(out=bcA.rearrange("p s b -> p (s b)"), in_=psum_bc)

        # ---------- deferred bulk DMA (sync ring, serial FIFO) ---------
        # Released once the summed logits landed (exchange complete): A
        # tail first so battery 0/1 combines stream just ahead of their
        # matmuls, then the remaining x batteries behind it on the ring.
        dtail = nc.sync.dma_start(out=A_sb[:, :, AEARLY, :],
                                  in_=am_ap[:, AEARLY, :, :])
        add_dep_helper(dtail.ins, lgt_dma.ins, sync=True,
                       reason="bulk DMA deferred until exchange completes")
        for kt in range(AEARLY + 1, 7):
            nc.sync.dma_start(out=A_sb[:, :, kt, :], in_=am_ap[:, kt, :, :])
        nc.sync.dma_start(out=A_sb[0:KREM, :, 7, :], in_=arem.ap())
        for b in range(1, BPC):
            load_xb(b)

        gps_ctx.close()
        w1_ctx.close()

        # ---------- main fused phase -----------------------------------
        mps = ctx.enter_context(tc.tile_pool(name="mpsum", bufs=2, space="PSUM"))
        wbpool = ctx.enter_context(tc.tile_pool(name="wbs", bufs=2))
        scpool = ctx.enter_context(tc.tile_pool(name="scratch", bufs=2))
        opool = ctx.enter_context(tc.tile_pool(name="outs", bufs=3))

        def _vload(eng, ap, name):
            reg = eng.alloc_register(name)
            eng.reg_load(reg, ap)
            val = eng.snap(reg, donate=True)
            return nc.s_assert_within(val, 0, E - 1, skip_runtime_assert=True)

        def combine(b, pieces=2):
            """wb = g1*A_e1 + g2*A_e2 for battery b, 2 fused ops per piece."""
            rv1 = _vload(nc.vector, sidx[b:b + 1, 0:1], f"e1_{b}")
            rv2 = _vload(nc.scalar, sidx[b:b + 1, 1:2], f"e2_{b}")
            wb = wbpool.tile([128, KT, D], BF16)
            w = KT // pieces
            for h in range(pieces):
                kts = slice(h * w, (h + 1) * w)
                t2 = scpool.tile([128, w, D], BF16, tag=f"t2_{pieces}")
                nc.scalar.activation(
                    out=t2.rearrange("p k d -> p (k d)"),
                    in_=A_sb[:, ds(rv2, 1), kts, :].rearrange("p o k d -> p (o k d)"),
                    func=mybir.ActivationFunctionType.Copy,
                    scale=bcA[:, 1, b:b + 1],
                )
                nc.vector.scalar_tensor_tensor(
                    out=wb[:, kts, :].rearrange("p k d -> p (k d)"),
                    in0=A_sb[:, ds(rv1, 1), kts, :].rearrange("p o k d -> p (o k d)"),
                    scalar=bcA[:, 0, b:b + 1],
                    in1=t2.rearrange("p k d -> p (k d)"),
                    op0=AluOpType.mult, op1=AluOpType.add,
                )
            return wb

        def battery(b, wb):
            xb = xb_tiles[b]
            pm = mps.tile([128, MT, D], F32, tag="mp")
            for kt in range(KT):
                np_ = KREM if kt == 7 else 128
                for m in range(MT):
                    nc.tensor.matmul(
                        out=pm[:, m, :],
                        lhsT=xb[0:np_, kt, ts(m, 128)],
                        rhs=wb[0:np_, kt, :],
                        start=(kt == 0), stop=(kt == KT - 1),
                    )
            osb = opool.tile([128, MT, D], BF16, tag="osb")
            nc.vector.tensor_copy(
                out=osb[:, 0:2, :].rearrange("p m d -> p (m d)"),
                in_=pm[:, 0:2, :].rearrange("p m d -> p (m d)"),
            )
            nc.scalar.activation(
                out=osb[:, 2:4, :].rearrange("p m d -> p (m d)"),
                in_=pm[:, 2:4, :].rearrange("p m d -> p (m d)"),
                func=mybir.ActivationFunctionType.Copy,
            )
            return nc.sync.dma_start(
                out=outd.ap()[b].rearrange("p m d -> p (m d)"),
                in_=osb.rearrange("p m d -> p (m d)"),
            )

        wbs = {0: combine(0, pieces=4), 1: combine(1, pieces=4)}
        for b in range(BPC):
            battery(b, wbs.pop(b))
            if b + 2 < BPC:
                wbs[b + 2] = combine(b + 2)

    # No epilogue needed: the exchange runs through the collectives
    # runtime (no kernel-managed semaphores to reset between executions).


def make_nc():
    from concourse import bacc
    nc = bacc.Bacc("TRN2", target_bir_lowering=False, debug=False,
                   num_devices=NCORES)
    build_program(nc)
    nc.finalize()
    return nc


def prep_inputs(cycle_curve_data, cycle_numbers, DKP_embeddings,
                gate_W1, gate_b1, gate_W2, gate_b2,
                expert_W, expert_b, gen_W, gen_b):
    """Host-side layout prep (reshape/pad/cast/weight-fold). Returns in_maps."""
    f32 = np.float32
    bf16 = ml_dtypes.bfloat16

    # fused expert weights A_e = gen_W + expert_W[e]; ones-row bias.
    A = np.empty((E, K, D), dtype=f32)
    A[:, :CF, :] = np.asarray(expert_W, dtype=f32) + np.asarray(gen_W, dtype=f32)
    A[:, CF, :] = np.asarray(expert_b, dtype=f32) + np.asarray(gen_b, dtype=f32)
    Abf = A.astype(bf16)
    # [128, 7(kt), E, D] so each k-tile is one contiguous DMA chunk.
    amain = np.ascontiguousarray(
        Abf[:, :896, :].reshape(E, 7, 128, D).transpose(2, 1, 0, 3))
    arem = np.ascontiguousarray(Abf[:, 896:K, :].transpose(1, 0, 2))

    # x transposed with ones-row, partition-major.
    x = np.asarray(cycle_curve_data, dtype=f32).reshape(B, L, CF)
    xT = np.empty((B, K, L), dtype=bf16)
    xT[:, :CF, :] = x.transpose(0, 2, 1).astype(bf16)
    xT[:, CF, :] = np.asarray(1.0, dtype=bf16)
    xmain = np.ascontiguousarray(
        xT[:, :896, :].reshape(B, 7, 128, L).transpose(0, 2, 1, 3))
    xrem = np.ascontiguousarray(xT[:, 896:K, :])

    # gating input, partition-major [128, 33*64].
    g = np.zeros((GK, B), dtype=f32)
    g[:DLLM, :] = np.asarray(DKP_embeddings, dtype=f32).T
    g[DLLM, :] = np.asarray(cycle_numbers, dtype=f32)[:, 0]
    g[DLLM + 1, :] = 1.0
    gintp = np.ascontiguousarray(
        g.reshape(GKT, 128, B).transpose(1, 0, 2).reshape(128, GKT * B)
        .astype(np.float16))

    W1p = np.zeros((GK, DFF), dtype=f32)
    W1p[:DLLM + 1, :] = np.asarray(gate_W1, dtype=f32)
    W1p[DLLM + 1, :] = np.asarray(gate_b1, dtype=f32)

    w2 = np.asarray(gate_W2, dtype=f32)
    # b2/8 as one row: every core accumulates it into its partial logits
    # via a K=1 matmul; the 8-core sum then includes b2 exactly once.
    b2rep = np.ascontiguousarray(
        np.asarray(gate_b2, dtype=f32).reshape(1, E) / NCORES)
    id64 = np.eye(B, dtype=f32)

    in_maps = []
    for c in range(NCORES):
        chunk = W1p[:, c * DFFC:(c + 1) * DFFC]
        w1pm = np.ascontiguousarray(
            chunk.reshape(GKT, 128, DFFC).transpose(1, 0, 2)
            .reshape(128, GKT * DFFC).astype(np.float16))
        w2pm = np.ascontiguousarray(
            w2[c * DFFC:(c + 1) * DFFC, :].reshape(2, 128, E)
            .transpose(1, 0, 2).reshape(128, 2 * E))
        sel = np.zeros((B, BPC), dtype=f32)
        for i in range(BPC):
            sel[c * BPC + i, i] = 1.0
        in_maps.append({
            "xmain": np.ascontiguousarray(xmain[c * BPC:(c + 1) * BPC]),
            "xrem": np.ascontiguousarray(xrem[c * BPC:(c + 1) * BPC]),
            "amain": amain,
            "arem": arem,
            "gintp": gintp,
            "w1": w1pm,
            "w2p": w2pm,
            "b2rep": b2rep,
            "selt": sel,
            "id64": id64,
        })
    return in_maps


_CACHED = {}


def run(inputs, trace=False, tmpdir=None):
    """Run on the 8 NeuronCores; returns (full_output, BassKernelResults)."""
    from concourse import bass_utils
    in_maps = prep_inputs(**inputs)
    nc = _CACHED.get("nc")
    if nc is None:
        nc = make_nc()
        _CACHED["nc"] = nc
    res = bass_utils.run_bass_kernel_spmd(
        nc, in_maps, core_ids=list(range(NCORES)), trace=trace, tmpdir=tmpdir
    )
    outs = [np.asarray(r["out"]) for r in res.results]
    full = np.concatenate(outs, axis=0)          # [B, 128, MT, D] bf16
    full = full.transpose(0, 2, 1, 3).reshape(B, L, D).astype(np.float32)
    return full, res


def kernel(**inputs):
    full, _ = run(inputs, trace=False)
    return full


# revision 30
# speedup vs baseline: 1.0078x; 1.0078x over previous
"""Trainium2 Bass kernel for FlattenIntraCycleMoELayer (top-2 MoE + general path).

Strategy (v3):
  - Data-parallel over B (8 batteries per core); gen_W folded into each
    expert on host (gates sum to 1): out = x @ (g1*A_e1 + g2*A_e2),
    A_e = gen_W + expert_W[e], bias folded via an appended ones-row.
  - Gating layer-1 d_ff-sharded; inputs loaded in interleaved chunks so
    L1 matmuls start ~3us in.  gelu shortened to a 5-op chain reading
    L1 PSUM directly; layer-2 runs on a zero-padded 128-row hT so the
    partial-logits eviction covers all 128 partitions (no memset) and
    adds gate_b2/8 in the same op (the 8-core sum then includes b2 once).
  - Cross-core logits exchange: the 7 remote_dma_broadcast descriptor
    writes are emitted at the TOP of the gpsimd stream (descriptor
    generation does not read data; both HW and the interp read the
    source at TRIGGER time), so only the cheap trigger waits for the
    partial-logits eviction.  The gang-launch dummy AllReduce comes
    after the trigger, where its ncfw barrier blocks only an idle gpsimd.
  - Post-exchange: 3-op tree sum -> one select matmul to my 8 batteries
    -> top-2 via g2 = sigmoid(l2 - l1), g1 = 1 - g2 (equivalent to the
    reference's masked-softmax renorm up to the 1e-9 eps) -> one 16-col
    broadcast matmul for g1/g2 across partitions.
  - Combine is 2 fused ops per piece: t2 = g2*A_e2 (ACT scale-copy),
    wb = (g1*A_e1) + t2 (DVE scalar_tensor_tensor).
  - DMA: A streamed in per-k-tile chunks on the scalar ring (issued at
    the top of the ACT stream), gating chunks + x batteries + outputs
    on the sync ring.
  - Main loop: kt-major matmuls per battery, combine one battery ahead,
    PSUM double-buffered, evictions batched 2 m-tiles per op (DVE/ACT).

Host-side prep only reshapes/pads/casts/re-parametrizes weights;
all model math runs on device.
"""

import numpy as np
import ml_dtypes


def _ensure_import_path():
    try:
        import concourse  # noqa: F401
    except ImportError:
        import sys
        for p in ("/opt/trn_rl_repo", "/root/.axon_site/_ro/trn_rl_repo"):
            if p not in sys.path:
                sys.path.insert(0, p)
        import concourse  # noqa: F401


_ensure_import_path()

import concourse.bass as bass  # noqa: E402
import concourse.tile as tile  # noqa: E402
from concourse import mybir  # noqa: E402
from concourse.bass import ds, ts  # noqa: E402
from concourse.alu_op_type import AluOpType  # noqa: E402
from concourse.tile import add_dep_helper  # noqa: E402

BF16 = mybir.dt.bfloat16
F32 = mybir.dt.float32
F16 = mybir.dt.float16
U32 = mybir.dt.uint32

# Problem shape constants (hardcoded per contest rules).
B, L, C, F = 64, 512, 3, 300
CF = C * F              # 900
K = CF + 1              # 901 contraction rows (data + ones row for bias)
KT = 8                  # k-tiles: 7 full + 1 remainder
KREM = K - 7 * 128      # 5 rows in the last k-tile
D = 512                 # d_model
E = 8                   # experts
NCORES = 8
BPC = B // NCORES       # 8 batteries per core
DLLM = 4096
GK = 4224               # padded gating contraction = 33*128
GKT = GK // 128         # 33
DFF = 2048
DFFC = DFF // NCORES    # 256 per-core d_ff chunk
MT = L // 128           # 4 m-tiles per battery
GCHUNKS = [(0, 9), (9, 21), (21, 33)]   # gating k-tile DMA chunks
NWARM = 10              # PE warm-up junk matmuls after the exchange


def build_program(nc):
    from contextlib import ExitStack

    xmain = nc.dram_tensor("xmain", [BPC, 128, 7, L], BF16, kind="ExternalInput")
    xrem = nc.dram_tensor("xrem", [BPC, KREM, L], BF16, kind="ExternalInput")
    amain = nc.dram_tensor("amain", [128, 7, E, D], BF16, kind="ExternalInput")
    arem = nc.dram_tensor("arem", [KREM, E, D], BF16, kind="ExternalInput")
    gintp = nc.dram_tensor("gintp", [128, GKT * B], F16, kind="ExternalInput")
    w1d = nc.dram_tensor("w1", [128, GKT * DFFC], F16, kind="ExternalInput")
    w2p = nc.dram_tensor("w2p", [128, 2 * E], F32, kind="ExternalInput")
    b2d = nc.dram_tensor("b2rep", [1, E], F32, kind="ExternalInput")
    seld = nc.dram_tensor("selt", [B, BPC], F32, kind="ExternalInput")
    id64d = nc.dram_tensor("id64", [B, B], F32, kind="ExternalInput")
    outd = nc.dram_tensor("out", [BPC, 128, MT, D], BF16, kind="ExternalOutput")

    with tile.TileContext(nc) as tc, ExitStack() as ctx:
        singles = ctx.enter_context(tc.tile_pool(name="singles", bufs=1))
        gpool = ctx.enter_context(tc.tile_pool(name="gate", bufs=1))
        dpool = ctx.enter_context(tc.tile_pool(name="dram", bufs=1, space="DRAM"))

        w1_ctx = ExitStack()
        w1pool = w1_ctx.enter_context(tc.tile_pool(name="w1s", bufs=1))
        gps_ctx = ExitStack()
        gps = gps_ctx.enter_context(tc.tile_pool(name="gpsum", bufs=1, space="PSUM"))

        # ---------- DMAs -----------------------------------------------
        # sync ring: gating input chunks + small tensors + battery-0 x;
        # scalar ring: w1 chunks, then the early A k-tiles.  Gating data
        # leads both rings so layer 1 can start ~13us in.
        ginT_sb = gpool.tile([128, GKT, B], F16)
        w1_sb = w1pool.tile([128, GKT, DFFC], F16)
        gin_ap = gintp.ap().rearrange("p (k b) -> p k b", k=GKT)
        w1_ap = w1d.ap().rearrange("p (k f) -> p k f", k=GKT)
        for (lo, hi) in GCHUNKS:
            nc.sync.dma_start(out=ginT_sb[:, lo:hi, :], in_=gin_ap[:, lo:hi, :])
            nc.scalar.dma_start(out=w1_sb[:, lo:hi, :], in_=w1_ap[:, lo:hi, :])
        w2_sb = gpool.tile([128, 2, E], F32)
        nc.sync.dma_start(out=w2_sb.rearrange("p j e -> p (j e)"), in_=w2p.ap())
        b2_sb = gpool.tile([1, E], F32)
        nc.sync.dma_start(out=b2_sb, in_=b2d.ap())
        sel_sb = gpool.tile([B, BPC], F32)
        nc.sync.dma_start(out=sel_sb, in_=seld.ap())
        id64 = gpool.tile([B, B], F32)
        nc.sync.dma_start(out=id64, in_=id64d.ap())

        # scalar ring: fused expert weights A, one chunk per k-tile so the
        # combine of piece kt only waits for chunk kt.  SBUF layout is
        # e-major (combine slices [e, kt-range, :] contiguously); the DMA
        # reads the contiguous kt-major host chunk and scatters per-e.
        # The AllReduce barrier absorbs core-launch skew, so all input can
        # stream during the wait: A k-tiles on the scalar ring, x on sync.
        A_sb = singles.tile([128, E, KT, D], BF16)
        am_ap = amain.ap()
        for kt in range(7):
            nc.scalar.dma_start(
                out=A_sb[:, :, kt, :],
                in_=am_ap[:, kt, :, :],
            )
        # zero k-tile 7 (rows KREM:128 are read by the combine), then land
        # the KREM data rows over it.
        nc.vector.memset(A_sb[:, :, 7, :], 0.0)
        nc.scalar.dma_start(out=A_sb[0:KREM, :, 7, :], in_=arem.ap())

        # sync ring: x batteries after the gating inputs.
        xmain_ap = xmain.ap()
        xrem_ap = xrem.ap()
        xb_tiles = [None] * BPC
        for b in range(BPC):
            xb = singles.tile([128, KT, L], BF16, tag=f"xb{b}")
            nc.sync.dma_start(
                out=xb[:, 0:7, :].rearrange("p k l -> p (k l)"),
                in_=xmain_ap[b].rearrange("p k l -> p (k l)"),
            )
            nc.sync.dma_start(out=xb[0:KREM, 7, :], in_=xrem_ap[b])
            xb_tiles[b] = xb

        # ---------- DVE constants / zero-pads (no deps, ~1us) ----------
        hT_sb = gpool.tile([128, 2, 128], F32)
        nc.vector.memset(hT_sb, 0.0)        # cols B:128 stay zero for L2
        ones_sb = gpool.tile([B, 128], F32)
        nc.vector.memset(ones_sb, 1.0)
        jt = gpool.tile([128, DFFC], F16, tag="junk")
        nc.vector.memset(jt, 0.0)

        # ---------- gating layer 1 (PE, chunk-ordered) -----------------
        # Pre-warm the PE while the first gating chunk is in flight: the
        # HAM clock-gate needs ~4us of sustained activity to unlock full
        # speed, and gating layer 1 is on the exchange-send critical path.
        psum_h = gps.tile([B, DFFC], F32, bufs=1)
        for j in range(16):
            nc.tensor.matmul(out=psum_h, lhsT=jt[:, 0:B], rhs=jt,
                             start=True, stop=True)
        for kt in range(GKT):
            nc.tensor.matmul(
                out=psum_h, lhsT=ginT_sb[:, kt, :], rhs=w1_sb[:, kt, :],
                start=(kt == 0), stop=(kt == GKT - 1),
            )

        # gelu (tanh approx):
        #   h = (0.5*x) * (1 + tanh(0.79788456*(x + 0.044715*x^3)))
        g_x = gpool.tile([B, DFFC], F32)
        nc.vector.tensor_copy(out=g_x, in_=psum_h)
        g_x2 = gpool.tile([B, DFFC], F32)
        nc.vector.tensor_tensor(out=g_x2, in0=g_x, in1=g_x, op=AluOpType.mult)
        g_xh = gpool.tile([B, DFFC], F32)
        nc.vector.tensor_scalar_mul(g_xh, g_x, 0.5)
        g_p = gpool.tile([B, DFFC], F32)
        nc.vector.tensor_scalar(g_p, g_x2, 0.044715, 1.0,
                                AluOpType.mult, AluOpType.add)
        g_u = gpool.tile([B, DFFC], F32)
        nc.vector.tensor_tensor(out=g_u, in0=g_x, in1=g_p, op=AluOpType.mult)
        g_t = gpool.tile([B, DFFC], F32)
        nc.scalar.activation(out=g_t, in_=g_u,
                             func=mybir.ActivationFunctionType.Tanh,
                             scale=0.7978845608028654)
        h_sb = gpool.tile([B, DFFC], F32)
        nc.vector.scalar_tensor_tensor(out=h_sb, in0=g_t, scalar=1.0, in1=g_xh,
                                       op0=AluOpType.add, op1=AluOpType.mult)

        # transpose h -> hT [128, 2, 0:B] (cols B:128 remain zero)
        for j in range(2):
            pst = gps.tile([128, B], F32, bufs=2, tag="pst")
            nc.tensor.transpose(
                out=pst, in_=h_sb[:, j * 128:(j + 1) * 128], identity=id64
            )
            nc.vector.tensor_copy(out=hT_sb[:, j, 0:B], in_=pst)

        # layer 2: partial logits on all 128 partitions; a third K=1
        # accumulation matmul (ones-row x b2/8-row) folds gate_b2/8 in,
        # so the 8-core sum includes b2 exactly once.
        psum_l = gps.tile([128, E], F32, bufs=2, tag="pst")
        for j in range(2):
            nc.tensor.matmul(out=psum_l, lhsT=hT_sb[:, j, :], rhs=w2_sb[:, j, :],
                             start=(j == 0), stop=False)
        nc.tensor.matmul(out=psum_l, lhsT=ones_sb[0:1, :], rhs=b2_sb,
                         start=False, stop=True)
        lgt_sb = gpool.tile([128, E], F32)
        nc.vector.tensor_copy(out=lgt_sb, in_=psum_l)

        # ---------- cross-core logits sum: real ncfw AllReduce ---------
        # The remote-DMA exchange path moves the 4KB payload as ~1000
        # 4-byte SWDGE packets (~25us serial per engine lane); the ncfw
        # AllReduce has a similar fixed latency but removes the Q7
        # descriptor-generation dispatch (~10us), the semaphore reset
        # epilogue, and all post-scheduling sem injection.  It also
        # registers the NEFF with the collectives runtime (gang launch).
        cc_in = dpool.tile([B, E], F32)
        nc.sync.dma_start(out=cc_in, in_=lgt_sb[0:B, :])
        cc_out = dpool.tile([B, E], F32, addr_space="Shared")
        nc.gpsimd.collective_compute(
            "AllReduce", AluOpType.add,
            replica_groups=[list(range(NCORES))],
            ins=[cc_in], outs=[cc_out],
        )
        logits_all = gpool.tile([B, E], F32)
        lgt_dma = nc.gpsimd.dma_start(out=logits_all, in_=cc_out[:, :])

        # select my 8 batteries (one-hot matmul); b2 already included
        psum_sel = gps.tile([BPC, E], F32, bufs=2, tag="pst")
        nc.tensor.matmul(out=psum_sel, lhsT=sel_sb, rhs=logits_all,
                         start=True, stop=True)
        logits_my = gpool.tile([BPC, E], F32)
        nc.vector.tensor_copy(out=logits_my, in_=psum_sel)

        # top-2 gates: g2 = sigmoid(l2 - l1), g1 = 1 - g2
        sorted8 = gpool.tile([BPC, E], F32)
        sidx = gpool.tile([BPC, E], U32)
        nc.vector.max(out=sorted8, in_=logits_my)
        nc.vector.max_index(out=sidx, in_max=sorted8, in_values=logits_my)
        diff = gpool.tile([BPC, 1], F32)
        nc.vector.tensor_tensor(out=diff, in0=sorted8[:, 1:2],
                                in1=sorted8[:, 0:1], op=AluOpType.subtract)
        g2c = gpool.tile([BPC, 1], F32)
        nc.scalar.activation(out=g2c, in_=diff,
                             func=mybir.ActivationFunctionType.Sigmoid,
                             scale=1.0)
        g1c = gpool.tile([BPC, 1], F32)
        nc.vector.tensor_scalar(g1c, g2c, -1.0, 1.0,
                                AluOpType.mult, AluOpType.add)

        # broadcast g1/g2 of each battery to all 128 partitions with one
        # matmul: rhs = [diag(g1) | diag(g2)] (8 x 16), lhsT = ones (8 x 128).
        rhs8 = gpool.tile([BPC, 2, BPC], F32)
        nc.vector.tensor_scalar_mul(rhs8[:, 0, :], id64[0:BPC, 0:BPC], g1c)
        nc.scalar.activation(out=rhs8[:, 1, :], in_=id64[0:BPC, 0:BPC],
                             func=mybir.ActivationFunctionType.Copy,
                             scale=g2c)
        psum_bc = gps.tile([128, 2 * BPC], F32, bufs=2, tag="pbc")
        nc.tensor.matmul(out=psum_bc, lhsT=ones_sb[0:BPC, :],
                         rhs=rhs8.rearrange("p s b -> p (s b)"),
                         start=True, stop=True)
        bcA = gpool.tile([128, 2, BPC], F32)
        nc.vector.tensor_copy(out=bcA.rearrange("p s b -> p (s b)"), in_=psum_bc)

        # PE warm-up after the (tiny) select/broadcast matmuls: re-ramps
        # the HAM clock-gate for the fused main-loop matmuls.
        for j in range(NWARM):
            nc.tensor.matmul(
                out=psum_h, lhsT=ginT_sb[:, j, :], rhs=w1_sb[:, j, :],
                start=True, stop=True,
            )

        gps_ctx.close()
        w1_ctx.close()

        # ---------- main fused phase -----------------------------------
        mps = ctx.enter_context(tc.tile_pool(name="mpsum", bufs=2, space="PSUM"))
        wbpool = ctx.enter_context(tc.tile_pool(name="wbs", bufs=2))
        scpool = ctx.enter_context(tc.tile_pool(name="scratch", bufs=2))
        opool = ctx.enter_context(tc.tile_pool(name="outs", bufs=3))

        def _vload(eng, ap, name):
            reg = eng.alloc_register(name)
            eng.reg_load(reg, ap)
            val = eng.snap(reg, donate=True)
            return nc.s_assert_within(val, 0, E - 1, skip_runtime_assert=True)

        def combine(b, pieces=2):
            """wb = g1*A_e1 + g2*A_e2 for battery b, 2 fused ops per piece."""
            rv1 = _vload(nc.vector, sidx[b:b + 1, 0:1], f"e1_{b}")
            rv2 = _vload(nc.scalar, sidx[b:b + 1, 1:2], f"e2_{b}")
            wb = wbpool.tile([128, KT, D], BF16)
            w = KT // pieces
            for h in range(pieces):
                kts = slice(h * w, (h + 1) * w)
                t2 = scpool.tile([128, w, D], BF16, tag=f"t2_{pieces}")
                nc.scalar.activation(
                    out=t2.rearrange("p k d -> p (k d)"),
                    in_=A_sb[:, ds(rv2, 1), kts, :].rearrange("p o k d -> p (o k d)"),
                    func=mybir.ActivationFunctionType.Copy,
                    scale=bcA[:, 1, b:b + 1],
                )
                nc.vector.scalar_tensor_tensor(
                    out=wb[:, kts, :].rearrange("p k d -> p (k d)"),
                    in0=A_sb[:, ds(rv1, 1), kts, :].rearrange("p o k d -> p (o k d)"),
                    scalar=bcA[:, 0, b:b + 1],
                    in1=t2.rearrange("p k d -> p (k d)"),
                    op0=AluOpType.mult, op1=AluOpType.add,
                )
            return wb

        def battery(b, wb):
            xb = xb_tiles[b]
            pm = mps.tile([128, MT, D], F32, tag="mp")
            for kt in range(KT):
                np_ = KREM if kt == 7 else 128
                for m in range(MT):
                    nc.tensor.matmul(
                        out=pm[:, m, :],
                        lhsT=xb[0:np_, kt, ts(m, 128)],
                        rhs=wb[0:np_, kt, :],
                        start=(kt == 0), stop=(kt == KT - 1),
                    )
            osb = opool.tile([128, MT, D], BF16, tag="osb")
            nc.vector.tensor_copy(
                out=osb[:, 0:2, :].rearrange("p m d -> p (m d)"),
                in_=pm[:, 0:2, :].rearrange("p m d -> p (m d)"),
            )
            nc.scalar.activation(
                out=osb[:, 2:4, :].rearrange("p m d -> p (m d)"),
                in_=pm[:, 2:4, :].rearrange("p m d -> p (m d)"),
                func=mybir.ActivationFunctionType.Copy,
            )
            return nc.sync.dma_start(
                out=outd.ap()[b].rearrange("p m d -> p (m d)"),
                in_=osb.rearrange("p m d -> p (m d)"),
            )

        wbs = {0: combine(0, pieces=4), 1: combine(1, pieces=4)}
        for b in range(BPC):
            battery(b, wbs.pop(b))
            if b + 2 < BPC:
                wbs[b + 2] = combine(b + 2)

    # No epilogue needed: the exchange runs through the collectives
    # runtime (no kernel-managed semaphores to reset between executions).


def make_nc():
    from concourse import bacc
    nc = bacc.Bacc("TRN2", target_bir_lowering=False, debug=False,
                   num_devices=NCORES)
    build_program(nc)
    nc.finalize()
    return nc


def prep_inputs(cycle_curve_data, cycle_numbers, DKP_embeddings,
                gate_W1, gate_b1, gate_W2, gate_b2,
                expert_W, expert_b, gen_W, gen_b):
    """Host-side layout prep (reshape/pad/cast/weight-fold). Returns in_maps."""
    f32 = np.float32
    bf16 = ml_dtypes.bfloat16

    # fused expert weights A_e = gen_W + expert_W[e]; ones-row bias.
    A = np.empty((E, K, D), dtype=f32)
    A[:, :CF, :] = np.asarray(expert_W, dtype=f32) + np.asarray(gen_W, dtype=f32)
    A[:, CF, :] = np.asarray(expert_b, dtype=f32) + np.asarray(gen_b, dtype=f32)
    Abf = A.astype(bf16)
    # [128, 7(kt), E, D] so each k-tile is one contiguous DMA chunk.
    amain = np.ascontiguousarray(
        Abf[:, :896, :].reshape(E, 7, 128, D).transpose(2, 1, 0, 3))
    arem = np.ascontiguousarray(Abf[:, 896:K, :].transpose(1, 0, 2))

    # x transposed with ones-row, partition-major.
    x = np.asarray(cycle_curve_data, dtype=f32).reshape(B, L, CF)
    xT = np.empty((B, K, L), dtype=bf16)
    xT[:, :CF, :] = x.transpose(0, 2, 1).astype(bf16)
    xT[:, CF, :] = np.asarray(1.0, dtype=bf16)
    xmain = np.ascontiguousarray(
        xT[:, :896, :].reshape(B, 7, 128, L).transpose(0, 2, 1, 3))
    xrem = np.ascontiguousarray(xT[:, 896:K, :])

    # gating input, partition-major [128, 33*64].
    g = np.zeros((GK, B), dtype=f32)
    g[:DLLM, :] = np.asarray(DKP_embeddings, dtype=f32).T
    g[DLLM, :] = np.asarray(cycle_numbers, dtype=f32)[:, 0]
    g[DLLM + 1, :] = 1.0
    gintp = np.ascontiguousarray(
        g.reshape(GKT, 128, B).transpose(1, 0, 2).reshape(128, GKT * B)
        .astype(np.float16))

    W1p = np.zeros((GK, DFF), dtype=f32)
    W1p[:DLLM + 1, :] = np.asarray(gate_W1, dtype=f32)
    W1p[DLLM + 1, :] = np.asarray(gate_b1, dtype=f32)

    w2 = np.asarray(gate_W2, dtype=f32)
    # b2/8 as one row: every core accumulates it into its partial logits
    # via a K=1 matmul; the 8-core sum then includes b2 exactly once.
    b2rep = np.ascontiguousarray(
        np.asarray(gate_b2, dtype=f32).reshape(1, E) / NCORES)
    id64 = np.eye(B, dtype=f32)

    in_maps = []
    for c in range(NCORES):
        chunk = W1p[:, c * DFFC:(c + 1) * DFFC]
        w1pm = np.ascontiguousarray(
            chunk.reshape(GKT, 128, DFFC).transpose(1, 0, 2)
            .reshape(128, GKT * DFFC).astype(np.float16))
        w2pm = np.ascontiguousarray(
            w2[c * DFFC:(c + 1) * DFFC, :].reshape(2, 128, E)
            .transpose(1, 0, 2).reshape(128, 2 * E))
        sel = np.zeros((B, BPC), dtype=f32)
        for i in range(BPC):
            sel[c * BPC + i, i] = 1.0
        in_maps.append({
            "xmain": np.ascontiguousarray(xmain[c * BPC:(c + 1) * BPC]),
            "xrem": np.ascontiguousarray(xrem[c * BPC:(c + 1) * BPC]),
            "amain": amain,
            "arem": arem,
            "gintp": gintp,
            "w1": w1pm,
            "w2p": w2pm,
            "b2rep": b2rep,
            "selt": sel,
            "id64": id64,
        })
    return in_maps


_CACHED = {}


def run(inputs, trace=False, tmpdir=None):
    """Run on the 8 NeuronCores; returns (full_output, BassKernelResults)."""
    from concourse import bass_utils
    in_maps = prep_inputs(**inputs)
    nc = _CACHED.get("nc")
    if nc is None:
        nc = make_nc()
        _CACHED["nc"] = nc
    res = bass_utils.run_bass_kernel_spmd(
        nc, in_maps, core_ids=list(range(NCORES)), trace=trace, tmpdir=tmpdir
    )
    outs = [np.asarray(r["out"]) for r in res.results]
    full = np.concatenate(outs, axis=0)          # [B, 128, MT, D] bf16
    full = full.transpose(0, 2, 1, 3).reshape(B, L, D).astype(np.float32)
    return full, res


def kernel(**inputs):
    full, _ = run(inputs, trace=False)
    return full


# revision 35
# speedup vs baseline: 1.0646x; 1.0563x over previous
"""Trainium2 Bass kernel for FlattenIntraCycleMoELayer (top-2 MoE + general path).

Strategy (v3):
  - Data-parallel over B (8 batteries per core); gen_W folded into each
    expert on host (gates sum to 1): out = x @ (g1*A_e1 + g2*A_e2),
    A_e = gen_W + expert_W[e], bias folded via an appended ones-row.
  - Gating layer-1 d_ff-sharded; inputs loaded in interleaved chunks so
    L1 matmuls start ~3us in.  gelu shortened to a 5-op chain reading
    L1 PSUM directly; layer-2 runs on a zero-padded 128-row hT so the
    partial-logits eviction covers all 128 partitions (no memset) and
    adds gate_b2/8 in the same op (the 8-core sum then includes b2 once).
  - Cross-core logits exchange: the 7 remote_dma_broadcast descriptor
    writes are emitted at the TOP of the gpsimd stream (descriptor
    generation does not read data; both HW and the interp read the
    source at TRIGGER time), so only the cheap trigger waits for the
    partial-logits eviction.  The gang-launch dummy AllReduce comes
    after the trigger, where its ncfw barrier blocks only an idle gpsimd.
  - Post-exchange: 3-op tree sum -> one select matmul to my 8 batteries
    -> top-2 via g2 = sigmoid(l2 - l1), g1 = 1 - g2 (equivalent to the
    reference's masked-softmax renorm up to the 1e-9 eps) -> one 16-col
    broadcast matmul for g1/g2 across partitions.
  - Combine is 2 fused ops per piece: t2 = g2*A_e2 (ACT scale-copy),
    wb = (g1*A_e1) + t2 (DVE scalar_tensor_tensor).
  - DMA: A streamed in per-k-tile chunks on the scalar ring (issued at
    the top of the ACT stream), gating chunks + x batteries + outputs
    on the sync ring.
  - Main loop: kt-major matmuls per battery, combine one battery ahead,
    PSUM double-buffered, evictions batched 2 m-tiles per op (DVE/ACT).

Host-side prep only reshapes/pads/casts/re-parametrizes weights;
all model math runs on device.
"""

import numpy as np
import ml_dtypes


def _ensure_import_path():
    try:
        import concourse  # noqa: F401
    except ImportError:
        import sys
        for p in ("/opt/trn_rl_repo", "/root/.axon_site/_ro/trn_rl_repo"):
            if p not in sys.path:
                sys.path.insert(0, p)
        import concourse  # noqa: F401


_ensure_import_path()

import concourse.bass as bass  # noqa: E402
import concourse.tile as tile  # noqa: E402
from concourse import mybir  # noqa: E402
from concourse.bass import ds, ts  # noqa: E402
from concourse.alu_op_type import AluOpType  # noqa: E402
from concourse.tile import add_dep_helper  # noqa: E402

BF16 = mybir.dt.bfloat16
F32 = mybir.dt.float32
F16 = mybir.dt.float16
U32 = mybir.dt.uint32

# Problem shape constants (hardcoded per contest rules).
B, L, C, F = 64, 512, 3, 300
CF = C * F              # 900
K = CF + 1              # 901 contraction rows (data + ones row for bias)
KT = 8                  # k-tiles: 7 full + 1 remainder
KREM = K - 7 * 128      # 5 rows in the last k-tile
D = 512                 # d_model
E = 8                   # experts
NCORES = 8
BPC = B // NCORES       # 8 batteries per core
DLLM = 4096
GK = 4224               # padded gating contraction = 33*128
GKT = GK // 128         # 33
DFF = 2048
DFFC = DFF // NCORES    # 256 per-core d_ff chunk
MT = L // 128           # 4 m-tiles per battery
GCHUNKS = [(0, 9), (9, 21), (21, 33)]   # gating k-tile DMA chunks
NWARM = 10              # PE warm-up junk matmuls after the exchange
_EARLY_CC = True        # enter the ncfw barrier at launch (HW path)


def build_program(nc):
    from contextlib import ExitStack

    xmain = nc.dram_tensor("xmain", [BPC, 128, 7, L], BF16, kind="ExternalInput")
    xrem = nc.dram_tensor("xrem", [BPC, KREM, L], BF16, kind="ExternalInput")
    amain = nc.dram_tensor("amain", [128, 7, E, D], BF16, kind="ExternalInput")
    arem = nc.dram_tensor("arem", [KREM, E, D], BF16, kind="ExternalInput")
    gintp = nc.dram_tensor("gintp", [128, GKT * B], F16, kind="ExternalInput")
    w1d = nc.dram_tensor("w1", [128, GKT * DFFC], F16, kind="ExternalInput")
    w2p = nc.dram_tensor("w2p", [128, 2 * E], F32, kind="ExternalInput")
    b2d = nc.dram_tensor("b2rep", [1, E], F32, kind="ExternalInput")
    seld = nc.dram_tensor("selt", [B, BPC], F32, kind="ExternalInput")
    id64d = nc.dram_tensor("id64", [B, B], F32, kind="ExternalInput")
    outd = nc.dram_tensor("out", [BPC, 128, MT, D], BF16, kind="ExternalOutput")

    with tile.TileContext(nc) as tc, ExitStack() as ctx:
        singles = ctx.enter_context(tc.tile_pool(name="singles", bufs=1))
        gpool = ctx.enter_context(tc.tile_pool(name="gate", bufs=1))
        dpool = ctx.enter_context(tc.tile_pool(name="dram", bufs=1, space="DRAM"))

        w1_ctx = ExitStack()
        w1pool = w1_ctx.enter_context(tc.tile_pool(name="w1s", bufs=1))
        gps_ctx = ExitStack()
        gps = gps_ctx.enter_context(tc.tile_pool(name="gpsum", bufs=1, space="PSUM"))

        # ---------- DMAs -----------------------------------------------
        # sync ring: gating input chunks + small tensors + battery-0 x;
        # scalar ring: w1 chunks, then the early A k-tiles.  Gating data
        # leads both rings so layer 1 can start ~13us in.
        ginT_sb = gpool.tile([128, GKT, B], F16)
        w1_sb = w1pool.tile([128, GKT, DFFC], F16)
        gin_ap = gintp.ap().rearrange("p (k b) -> p k b", k=GKT)
        w1_ap = w1d.ap().rearrange("p (k f) -> p k f", k=GKT)
        for (lo, hi) in GCHUNKS:
            nc.sync.dma_start(out=ginT_sb[:, lo:hi, :], in_=gin_ap[:, lo:hi, :])
            nc.scalar.dma_start(out=w1_sb[:, lo:hi, :], in_=w1_ap[:, lo:hi, :])
        w2_sb = gpool.tile([128, 2, E], F32)
        nc.sync.dma_start(out=w2_sb.rearrange("p j e -> p (j e)"), in_=w2p.ap())
        b2_sb = gpool.tile([1, E], F32)
        nc.sync.dma_start(out=b2_sb, in_=b2d.ap())
        sel_sb = gpool.tile([B, BPC], F32)
        nc.sync.dma_start(out=sel_sb, in_=seld.ap())
        id64 = gpool.tile([B, B], F32)
        nc.sync.dma_start(out=id64, in_=id64d.ap())

        # scalar ring: fused expert weights A, one chunk per k-tile so the
        # combine of piece kt only waits for chunk kt.  SBUF layout is
        # e-major (combine slices [e, kt-range, :] contiguously); the DMA
        # reads the contiguous kt-major host chunk and scatters per-e.
        # The AllReduce barrier absorbs core-launch skew, so all input can
        # stream during the wait: A k-tiles on the scalar ring, x on sync.
        A_sb = singles.tile([128, E, KT, D], BF16)
        am_ap = amain.ap()
        for kt in range(7):
            nc.scalar.dma_start(
                out=A_sb[:, :, kt, :],
                in_=am_ap[:, kt, :, :],
            )
        # zero k-tile 7 (rows KREM:128 are read by the combine), then land
        # the KREM data rows over it.
        nc.vector.memset(A_sb[:, :, 7, :], 0.0)
        nc.scalar.dma_start(out=A_sb[0:KREM, :, 7, :], in_=arem.ap())

        # sync ring: x batteries after the gating inputs.
        xmain_ap = xmain.ap()
        xrem_ap = xrem.ap()
        xb_tiles = [None] * BPC
        for b in range(BPC):
            xb = singles.tile([128, KT, L], BF16, tag=f"xb{b}")
            nc.sync.dma_start(
                out=xb[:, 0:7, :].rearrange("p k l -> p (k l)"),
                in_=xmain_ap[b].rearrange("p k l -> p (k l)"),
            )
            nc.sync.dma_start(out=xb[0:KREM, 7, :], in_=xrem_ap[b])
            xb_tiles[b] = xb

        # ---------- DVE constants / zero-pads (no deps, ~1us) ----------
        hT_sb = gpool.tile([128, 2, 128], F32)
        nc.vector.memset(hT_sb, 0.0)        # cols B:128 stay zero for L2
        ones_sb = gpool.tile([B, 128], F32)
        nc.vector.memset(ones_sb, 1.0)
        jt = gpool.tile([128, DFFC], F16, tag="junk")
        nc.vector.memset(jt, 0.0)

        # ---------- gating layer 1 (PE, chunk-ordered) -----------------
        # Pre-warm the PE while the first gating chunk is in flight: the
        # HAM clock-gate needs ~4us of sustained activity to unlock full
        # speed, and gating layer 1 is on the exchange-send critical path.
        psum_h = gps.tile([B, DFFC], F32, bufs=1)
        for j in range(16):
            nc.tensor.matmul(out=psum_h, lhsT=jt[:, 0:B], rhs=jt,
                             start=True, stop=True)
        for kt in range(GKT):
            nc.tensor.matmul(
                out=psum_h, lhsT=ginT_sb[:, kt, :], rhs=w1_sb[:, kt, :],
                start=(kt == 0), stop=(kt == GKT - 1),
            )

        # gelu (tanh approx):
        #   h = (0.5*x) * (1 + tanh(0.79788456*(x + 0.044715*x^3)))
        g_x = gpool.tile([B, DFFC], F32)
        nc.vector.tensor_copy(out=g_x, in_=psum_h)
        g_x2 = gpool.tile([B, DFFC], F32)
        nc.vector.tensor_tensor(out=g_x2, in0=g_x, in1=g_x, op=AluOpType.mult)
        g_xh = gpool.tile([B, DFFC], F32)
        nc.vector.tensor_scalar_mul(g_xh, g_x, 0.5)
        g_p = gpool.tile([B, DFFC], F32)
        nc.vector.tensor_scalar(g_p, g_x2, 0.044715, 1.0,
                                AluOpType.mult, AluOpType.add)
        g_u = gpool.tile([B, DFFC], F32)
        nc.vector.tensor_tensor(out=g_u, in0=g_x, in1=g_p, op=AluOpType.mult)
        g_t = gpool.tile([B, DFFC], F32)
        nc.scalar.activation(out=g_t, in_=g_u,
                             func=mybir.ActivationFunctionType.Tanh,
                             scale=0.7978845608028654)
        h_sb = gpool.tile([B, DFFC], F32)
        nc.vector.scalar_tensor_tensor(out=h_sb, in0=g_t, scalar=1.0, in1=g_xh,
                                       op0=AluOpType.add, op1=AluOpType.mult)

        # transpose h -> hT [128, 2, 0:B] (cols B:128 remain zero)
        for j in range(2):
            pst = gps.tile([128, B], F32, bufs=2, tag="pst")
            nc.tensor.transpose(
                out=pst, in_=h_sb[:, j * 128:(j + 1) * 128], identity=id64
            )
            nc.vector.tensor_copy(out=hT_sb[:, j, 0:B], in_=pst)

        # layer 2: partial logits on all 128 partitions; a third K=1
        # accumulation matmul (ones-row x b2/8-row) folds gate_b2/8 in,
        # so the 8-core sum includes b2 exactly once.
        psum_l = gps.tile([128, E], F32, bufs=2, tag="pst")
        for j in range(2):
            nc.tensor.matmul(out=psum_l, lhsT=hT_sb[:, j, :], rhs=w2_sb[:, j, :],
                             start=(j == 0), stop=False)
        nc.tensor.matmul(out=psum_l, lhsT=ones_sb[0:1, :], rhs=b2_sb,
                         start=False, stop=True)
        lgt_sb = gpool.tile([128, E], F32)
        nc.vector.tensor_copy(out=lgt_sb, in_=psum_l)

        # ---------- cross-core logits sum: real ncfw AllReduce ---------
        # The remote-DMA exchange path moves the 4KB payload as ~1000
        # 4-byte SWDGE packets (~25us serial per engine lane); the ncfw
        # AllReduce has a similar fixed latency but removes the Q7
        # descriptor-generation dispatch (~10us), the semaphore reset
        # epilogue, and all post-scheduling sem injection.  It also
        # registers the NEFF with the collectives runtime (gang launch).
        cc_in = dpool.tile([B, E], F32)
        nc.scalar.dma_start(out=cc_in, in_=lgt_sb[0:B, :])
        cc_out = dpool.tile([B, E], F32, addr_space="Shared")
        cc = nc.gpsimd.collective_compute(
            "AllReduce", AluOpType.add,
            replica_groups=[list(range(NCORES))],
            ins=[cc_in], outs=[cc_out],
        )
        logits_all = gpool.tile([B, E], F32)
        lgt_dma = nc.gpsimd.dma_start(out=logits_all, in_=cc_out[:, :])

        # select my 8 batteries (one-hot matmul); b2 already included
        psum_sel = gps.tile([BPC, E], F32, bufs=2, tag="pst")
        nc.tensor.matmul(out=psum_sel, lhsT=sel_sb, rhs=logits_all,
                         start=True, stop=True)
        logits_my = gpool.tile([BPC, E], F32)
        nc.vector.tensor_copy(out=logits_my, in_=psum_sel)

        # top-2 gates: g2 = sigmoid(l2 - l1), g1 = 1 - g2
        sorted8 = gpool.tile([BPC, E], F32)
        sidx = gpool.tile([BPC, E], U32)
        nc.vector.max(out=sorted8, in_=logits_my)
        nc.vector.max_index(out=sidx, in_max=sorted8, in_values=logits_my)
        diff = gpool.tile([BPC, 1], F32)
        nc.vector.tensor_tensor(out=diff, in0=sorted8[:, 1:2],
                                in1=sorted8[:, 0:1], op=AluOpType.subtract)
        g2c = gpool.tile([BPC, 1], F32)
        nc.scalar.activation(out=g2c, in_=diff,
                             func=mybir.ActivationFunctionType.Sigmoid,
                             scale=1.0)
        g1c = gpool.tile([BPC, 1], F32)
        nc.vector.tensor_scalar(g1c, g2c, -1.0, 1.0,
                                AluOpType.mult, AluOpType.add)

        # broadcast g1/g2 of each battery to all 128 partitions with one
        # matmul: rhs = [diag(g1) | diag(g2)] (8 x 16), lhsT = ones (8 x 128).
        rhs8 = gpool.tile([BPC, 2, BPC], F32)
        nc.vector.tensor_scalar_mul(rhs8[:, 0, :], id64[0:BPC, 0:BPC], g1c)
        nc.scalar.activation(out=rhs8[:, 1, :], in_=id64[0:BPC, 0:BPC],
                             func=mybir.ActivationFunctionType.Copy,
                             scale=g2c)
        psum_bc = gps.tile([128, 2 * BPC], F32, bufs=2, tag="pbc")
        nc.tensor.matmul(out=psum_bc, lhsT=ones_sb[0:BPC, :],
                         rhs=rhs8.rearrange("p s b -> p (s b)"),
                         start=True, stop=True)
        bcA = gpool.tile([128, 2, BPC], F32)
        nc.vector.tensor_copy(out=bcA.rearrange("p s b -> p (s b)"), in_=psum_bc)

        # PE warm-up gated on the logits arrival: re-ramps the HAM
        # clock-gate for the fused main-loop matmuls (without the gate the
        # scheduler hoists these into the barrier wait, where they ramp
        # nothing).
        for j in range(NWARM):
            jmm = nc.tensor.matmul(
                out=psum_h, lhsT=ginT_sb[:, j, :], rhs=w1_sb[:, j, :],
                start=True, stop=True,
            )
            if j == 0:
                add_dep_helper(jmm.ins, lgt_dma.ins, sync=True,
                               reason="warm-up starts when logits land")

        gps_ctx.close()
        w1_ctx.close()

        # ---------- main fused phase -----------------------------------
        mps = ctx.enter_context(tc.tile_pool(name="mpsum", bufs=2, space="PSUM"))
        wbpool = ctx.enter_context(tc.tile_pool(name="wbs", bufs=2))
        scpool = ctx.enter_context(tc.tile_pool(name="scratch", bufs=2))
        opool = ctx.enter_context(tc.tile_pool(name="outs", bufs=3))

        def _vload(eng, ap, name):
            reg = eng.alloc_register(name)
            eng.reg_load(reg, ap)
            val = eng.snap(reg, donate=True)
            return nc.s_assert_within(val, 0, E - 1, skip_runtime_assert=True)

        def combine(b, pieces=2):
            """wb = g1*A_e1 + g2*A_e2 for battery b, 2 fused ops per piece."""
            rv1 = _vload(nc.vector, sidx[b:b + 1, 0:1], f"e1_{b}")
            rv2 = _vload(nc.scalar, sidx[b:b + 1, 1:2], f"e2_{b}")
            wb = wbpool.tile([128, KT, D], BF16)
            w = KT // pieces
            for h in range(pieces):
                kts = slice(h * w, (h + 1) * w)
                t2 = scpool.tile([128, w, D], BF16, tag=f"t2_{pieces}")
                nc.scalar.activation(
                    out=t2.rearrange("p k d -> p (k d)"),
                    in_=A_sb[:, ds(rv2, 1), kts, :].rearrange("p o k d -> p (o k d)"),
                    func=mybir.ActivationFunctionType.Copy,
                    scale=bcA[:, 1, b:b + 1],
                )
                nc.vector.scalar_tensor_tensor(
                    out=wb[:, kts, :].rearrange("p k d -> p (k d)"),
                    in0=A_sb[:, ds(rv1, 1), kts, :].rearrange("p o k d -> p (o k d)"),
                    scalar=bcA[:, 0, b:b + 1],
                    in1=t2.rearrange("p k d -> p (k d)"),
                    op0=AluOpType.mult, op1=AluOpType.add,
                )
            return wb

        def battery(b, wb):
            xb = xb_tiles[b]
            pm = mps.tile([128, MT, D], F32, tag="mp")
            for kt in range(KT):
                np_ = KREM if kt == 7 else 128
                for m in range(MT):
                    nc.tensor.matmul(
                        out=pm[:, m, :],
                        lhsT=xb[0:np_, kt, ts(m, 128)],
                        rhs=wb[0:np_, kt, :],
                        start=(kt == 0), stop=(kt == KT - 1),
                    )
            osb = opool.tile([128, MT, D], BF16, tag="osb")
            nc.vector.tensor_copy(
                out=osb[:, 0:2, :].rearrange("p m d -> p (m d)"),
                in_=pm[:, 0:2, :].rearrange("p m d -> p (m d)"),
            )
            nc.scalar.activation(
                out=osb[:, 2:4, :].rearrange("p m d -> p (m d)"),
                in_=pm[:, 2:4, :].rearrange("p m d -> p (m d)"),
                func=mybir.ActivationFunctionType.Copy,
            )
            return nc.sync.dma_start(
                out=outd.ap()[b].rearrange("p m d -> p (m d)"),
                in_=osb.rearrange("p m d -> p (m d)"),
            )

        wbs = {0: combine(0, pieces=4), 1: combine(1, pieces=4)}
        for b in range(BPC):
            battery(b, wbs.pop(b))
            if b + 2 < BPC:
                wbs[b + 2] = combine(b + 2)

    # No epilogue needed: the exchange runs through the collectives
    # runtime (no kernel-managed semaphores to reset between executions).

    # Post-scheduling: strip the collective's scheduled waits so every
    # core enters the ncfw barrier immediately at launch.  The barrier
    # handshake costs ~55us of firmware time, which then overlaps the
    # gating phase; the reduce pipeline only reads cc_in after the
    # barrier releases, ~30us after the partial logits land there.
    # (The sim keeps the waits — set kernel._EARLY_CC = False before
    # make_nc() — because the interp reads cc_in at instruction entry.)
    if _EARLY_CC:
        cc.ins.sync_info.on_wait.clear()


def make_nc():
    from concourse import bacc
    nc = bacc.Bacc("TRN2", target_bir_lowering=False, debug=False,
                   num_devices=NCORES)
    build_program(nc)
    nc.finalize()
    return nc


def prep_inputs(cycle_curve_data, cycle_numbers, DKP_embeddings,
                gate_W1, gate_b1, gate_W2, gate_b2,
                expert_W, expert_b, gen_W, gen_b):
    """Host-side layout prep (reshape/pad/cast/weight-fold). Returns in_maps."""
    f32 = np.float32
    bf16 = ml_dtypes.bfloat16

    # fused expert weights A_e = gen_W + expert_W[e]; ones-row bias.
    A = np.empty((E, K, D), dtype=f32)
    A[:, :CF, :] = np.asarray(expert_W, dtype=f32) + np.asarray(gen_W, dtype=f32)
    A[:, CF, :] = np.asarray(expert_b, dtype=f32) + np.asarray(gen_b, dtype=f32)
    Abf = A.astype(bf16)
    # [128, 7(kt), E, D] so each k-tile is one contiguous DMA chunk.
    amain = np.ascontiguousarray(
        Abf[:, :896, :].reshape(E, 7, 128, D).transpose(2, 1, 0, 3))
    arem = np.ascontiguousarray(Abf[:, 896:K, :].transpose(1, 0, 2))

    # x transposed with ones-row, partition-major.
    x = np.asarray(cycle_curve_data, dtype=f32).reshape(B, L, CF)
    xT = np.empty((B, K, L), dtype=bf16)
    xT[:, :CF, :] = x.transpose(0, 2, 1).astype(bf16)
    xT[:, CF, :] = np.asarray(1.0, dtype=bf16)
    xmain = np.ascontiguousarray(
        xT[:, :896, :].reshape(B, 7, 128, L).transpose(0, 2, 1, 3))
    xrem = np.ascontiguousarray(xT[:, 896:K, :])

    # gating input, partition-major [128, 33*64].
    g = np.zeros((GK, B), dtype=f32)
    g[:DLLM, :] = np.asarray(DKP_embeddings, dtype=f32).T
    g[DLLM, :] = np.asarray(cycle_numbers, dtype=f32)[:, 0]
    g[DLLM + 1, :] = 1.0
    gintp = np.ascontiguousarray(
        g.reshape(GKT, 128, B).transpose(1, 0, 2).reshape(128, GKT * B)
        .astype(np.float16))

    W1p = np.zeros((GK, DFF), dtype=f32)
    W1p[:DLLM + 1, :] = np.asarray(gate_W1, dtype=f32)
    W1p[DLLM + 1, :] = np.asarray(gate_b1, dtype=f32)

    w2 = np.asarray(gate_W2, dtype=f32)
    # b2/8 as one row: every core accumulates it into its partial logits
    # via a K=1 matmul; the 8-core sum then includes b2 exactly once.
    b2rep = np.ascontiguousarray(
        np.asarray(gate_b2, dtype=f32).reshape(1, E) / NCORES)
    id64 = np.eye(B, dtype=f32)

    in_maps = []
    for c in range(NCORES):
        chunk = W1p[:, c * DFFC:(c + 1) * DFFC]
        w1pm = np.ascontiguousarray(
            chunk.reshape(GKT, 128, DFFC).transpose(1, 0, 2)
            .reshape(128, GKT * DFFC).astype(np.float16))
        w2pm = np.ascontiguousarray(
            w2[c * DFFC:(c + 1) * DFFC, :].reshape(2, 128, E)
            .transpose(1, 0, 2).reshape(128, 2 * E))
        sel = np.zeros((B, BPC), dtype=f32)
        for i in range(BPC):
            sel[c * BPC + i, i] = 1.0
        in_maps.append({
            "xmain": np.ascontiguousarray(xmain[c * BPC:(c + 1) * BPC]),
            "xrem": np.ascontiguousarray(xrem[c * BPC:(c + 1) * BPC]),
            "amain": amain,
            "arem": arem,
            "gintp": gintp,
            "w1": w1pm,
            "w2p": w2pm,
            "b2rep": b2rep,
            "selt": sel,
            "id64": id64,
        })
    return in_maps


_CACHED = {}


def run(inputs, trace=False, tmpdir=None):
    """Run on the 8 NeuronCores; returns (full_output, BassKernelResults)."""
    from concourse import bass_utils
    in_maps = prep_inputs(**inputs)
    nc = _CACHED.get("nc")
    if nc is None:
        nc = make_nc()
        _CACHED["nc"] = nc
    res = bass_utils.run_bass_kernel_spmd(
        nc, in_maps, core_ids=list(range(NCORES)), trace=trace, tmpdir=tmpdir
    )
    outs = [np.asarray(r["out"]) for r in res.results]
    full = np.concatenate(outs, axis=0)          # [B, 128, MT, D] bf16
    full = full.transpose(0, 2, 1, 3).reshape(B, L, D).astype(np.float32)
    return full, res


def kernel(**inputs):
    full, _ = run(inputs, trace=False)
    return full


# revision 37
# speedup vs baseline: 1.1378x; 1.0687x over previous
"""Trainium2 Bass kernel for FlattenIntraCycleMoELayer (top-2 MoE + general path).

Strategy (v4 — fully local, no cross-core communication):
  - Data-parallel over B (8 batteries per core); gen_W folded into each
    expert on host (gates sum to 1): out = x @ (g1*A_e1 + g2*A_e2),
    A_e = gen_W + expert_W[e], bias folded via an appended ones-row.
  - The gating network is REPLICATED per core for its own 8 batteries:
    the full W1 (17.3MB f16) is streamed through a 4-deep chunk pool
    and consumed by layer-1 matmuls as it lands.  This removes every
    cross-core dependency: the d_ff-sharded alternative needs a logits
    exchange, and both available transports cost ~55-80us of fixed
    latency (ncfw barrier handshake ~70us; SWDGE remote-DMA moves the
    payload as ~1000 4-byte packets plus ~30us of Q7 dispatch+trigger
    latency).  Streaming 17.3MB at ~350GB/s costs ~50us and overlaps
    the A/x loads, is deterministic, and is immune to launch skew.
  - Gating math: L1 accumulates into a [8, 4x512] PSUM (batteries on
    partitions), gelu + 16 PE transposes produce hT [128, 16, 8], L2 is
    16 tiny matmuls + a K=1 ones-row matmul folding gate_b2; top-2 via
    g2 = sigmoid(l2 - l1), g1 = 1 - g2 (equivalent to the reference's
    masked-softmax renorm up to the 1e-9 eps); one 16-col broadcast
    matmul replicates g1/g2 across partitions.
  - Combine is 2 fused ops per piece: t2 = g2*A_e2 (ACT scale-copy),
    wb = (g1*A_e1) + t2 (DVE scalar_tensor_tensor).
  - Main loop: kt-major matmuls per battery, combine one battery ahead,
    PSUM double-buffered, evictions batched 2 m-tiles per op (DVE/ACT).

Host-side prep only reshapes/pads/casts/re-parametrizes weights;
all model math runs on device.
"""

import numpy as np
import ml_dtypes


def _ensure_import_path():
    try:
        import concourse  # noqa: F401
    except ImportError:
        import sys
        for p in ("/opt/trn_rl_repo", "/root/.axon_site/_ro/trn_rl_repo"):
            if p not in sys.path:
                sys.path.insert(0, p)
        import concourse  # noqa: F401


_ensure_import_path()

import concourse.bass as bass  # noqa: E402
import concourse.tile as tile  # noqa: E402
from concourse import mybir  # noqa: E402
from concourse.bass import ds, ts  # noqa: E402
from concourse.alu_op_type import AluOpType  # noqa: E402
from concourse.tile import add_dep_helper  # noqa: E402

BF16 = mybir.dt.bfloat16
F32 = mybir.dt.float32
F16 = mybir.dt.float16
U32 = mybir.dt.uint32

# Problem shape constants (hardcoded per contest rules).
B, L, C, F = 64, 512, 3, 300
CF = C * F              # 900
K = CF + 1              # 901 contraction rows (data + ones row for bias)
KT = 8                  # k-tiles: 7 full + 1 remainder
KREM = K - 7 * 128      # 5 rows in the last k-tile
D = 512                 # d_model
E = 8                   # experts
NCORES = 8
BPC = B // NCORES       # 8 batteries per core
DLLM = 4096
GK = 4224               # padded gating contraction = 33*128
GKT = GK // 128         # 33
DFF = 2048
DFT = DFF // 128        # 16 d_ff tiles of 128
MT = L // 128           # 4 m-tiles per battery
WCH = 3                 # w1 k-tiles per streamed chunk
NCH = 11                # 11 chunks of 3 k-tiles = 33
NWARM = 16              # PE warm-up junk matmuls before the main loop


def build_program(nc):
    from contextlib import ExitStack

    xmain = nc.dram_tensor("xmain", [BPC, 128, 7, L], BF16, kind="ExternalInput")
    xrem = nc.dram_tensor("xrem", [BPC, KREM, L], BF16, kind="ExternalInput")
    amain = nc.dram_tensor("amain", [128, 7, E, D], BF16, kind="ExternalInput")
    arem = nc.dram_tensor("arem", [KREM, E, D], BF16, kind="ExternalInput")
    ginp = nc.dram_tensor("ginp", [128, GKT * BPC], F16, kind="ExternalInput")
    w1d = nc.dram_tensor("w1f", [128, GKT * DFF], F16, kind="ExternalInput")
    w2p = nc.dram_tensor("w2f", [128, DFT * E], F32, kind="ExternalInput")
    b2d = nc.dram_tensor("b2row", [1, E], F32, kind="ExternalInput")
    id64d = nc.dram_tensor("id64", [B, B], F32, kind="ExternalInput")
    outd = nc.dram_tensor("out", [BPC, 128, MT, D], BF16, kind="ExternalOutput")

    with tile.TileContext(nc) as tc, ExitStack() as ctx:
        singles = ctx.enter_context(tc.tile_pool(name="singles", bufs=1))
        gpool = ctx.enter_context(tc.tile_pool(name="gate", bufs=1))

        w1_ctx = ExitStack()
        w1pool = w1_ctx.enter_context(tc.tile_pool(name="w1s", bufs=4))
        gps_ctx = ExitStack()
        gps = gps_ctx.enter_context(tc.tile_pool(name="gpsum", bufs=1, space="PSUM"))

        # ---------- DMAs -----------------------------------------------
        # w1 chunks alternate between the two HWDGE rings (sync/scalar);
        # the 4-deep pool keeps issues ahead of consumption.  A and x are
        # interleaved behind the w1 stream on both rings.
        ginT_sb = gpool.tile([128, GKT, BPC], F16)
        nc.sync.dma_start(out=ginT_sb.rearrange("p k b -> p (k b)"), in_=ginp.ap())
        w2_sb = gpool.tile([128, DFT, E], F32)
        nc.sync.dma_start(out=w2_sb.rearrange("p j e -> p (j e)"), in_=w2p.ap())
        b2_sb = gpool.tile([1, E], F32)
        nc.sync.dma_start(out=b2_sb, in_=b2d.ap())
        id64 = gpool.tile([B, B], F32)
        nc.sync.dma_start(out=id64, in_=id64d.ap())

        w1_ap = w1d.ap().rearrange("p (k f) -> p k f", k=GKT)
        w1c_tiles = []
        for ci in range(NCH):
            eng = nc.sync if ci % 2 == 0 else nc.scalar
            w1c = w1pool.tile([128, WCH, DFF], F16, tag="w1c")
            eng.dma_start(
                out=w1c.rearrange("p k f -> p (k f)"),
                in_=w1_ap[:, ci * WCH:(ci + 1) * WCH, :].rearrange(
                    "p k f -> p (k f)"),
            )
            w1c_tiles.append(w1c)

        # x battery 0 early on sync (needed at main-loop start).
        xmain_ap = xmain.ap()
        xrem_ap = xrem.ap()
        xb_tiles = [None] * BPC

        def load_xb(b, eng):
            xb = xb_tiles[b]
            eng.dma_start(
                out=xb[:, 0:7, :].rearrange("p k l -> p (k l)"),
                in_=xmain_ap[b].rearrange("p k l -> p (k l)"),
            )
            eng.dma_start(out=xb[0:KREM, 7, :], in_=xrem_ap[b])

        for b in range(BPC):
            xb = singles.tile([128, KT, L], BF16, tag=f"xb{b}")
            xb_tiles[b] = xb
        load_xb(0, nc.sync)

        # A: even k-tiles on sync, odd + remainder on scalar.
        A_sb = singles.tile([128, E, KT, D], BF16)
        am_ap = amain.ap()
        nc.vector.memset(A_sb[:, :, 7, :], 0.0)
        for kt in (1, 3, 5):
            nc.scalar.dma_start(out=A_sb[:, :, kt, :], in_=am_ap[:, kt, :, :])
        nc.scalar.dma_start(out=A_sb[0:KREM, :, 7, :], in_=arem.ap())
        for kt in (0, 2, 4, 6):
            nc.sync.dma_start(out=A_sb[:, :, kt, :], in_=am_ap[:, kt, :, :])
        for b in (1, 2, 3):
            load_xb(b, nc.sync)
        for b in (4, 5, 6, 7):
            load_xb(b, nc.scalar)

        # ---------- DVE constants (no deps, ~1us) ----------------------
        ones_sb = gpool.tile([BPC, 128], F32)
        nc.vector.memset(ones_sb, 1.0)
        jt = gpool.tile([128, 512], F16, tag="junk")
        nc.vector.memset(jt, 0.0)

        # ---------- gating layer 1 (PE, chunk-streamed) ----------------
        # psum_h[b, nb*512 + f] accumulates over all 33 k-tiles.
        psum_h = gps.tile([BPC, 4, 512], F32, bufs=1)
        for ci in range(NCH):
            for kl in range(WCH):
                gkt = ci * WCH + kl
                for nb in range(4):
                    nc.tensor.matmul(
                        out=psum_h[:, nb, :],
                        lhsT=ginT_sb[:, gkt, :],
                        rhs=w1c_tiles[ci][:, kl, nb * 512:(nb + 1) * 512],
                        start=(gkt == 0), stop=(gkt == GKT - 1),
                    )

        # transpose the pre-activation first (gelu is pointwise and
        # commutes with the transpose): hX [128, 16, 8] keeps the gelu
        # intermediates at 512B/partition instead of 8KB.
        h8 = gpool.tile([BPC, DFF], F32)
        nc.vector.tensor_copy(out=h8, in_=psum_h.rearrange("p n f -> p (n f)"))
        hX = gpool.tile([128, DFT, BPC], F32)
        for j in range(DFT):
            pst = gps.tile([128, BPC], F32, bufs=2, tag="pst")
            nc.tensor.transpose(
                out=pst, in_=h8[:, j * 128:(j + 1) * 128],
                identity=id64[0:BPC, 0:BPC],
            )
            if j % 2 == 0:
                nc.vector.tensor_copy(out=hX[:, j, :], in_=pst)
            else:
                nc.scalar.activation(out=hX[:, j, :], in_=pst,
                                     func=mybir.ActivationFunctionType.Copy)

        # gelu (tanh approx) on the transposed layout:
        #   h = (0.5*x) * (1 + tanh(0.79788456*(x + 0.044715*x^3)))
        hXf = hX.rearrange("p j b -> p (j b)")
        g_x2 = gpool.tile([128, DFT * BPC], F32)
        nc.vector.tensor_tensor(out=g_x2, in0=hXf, in1=hXf, op=AluOpType.mult)
        g_xh = gpool.tile([128, DFT * BPC], F32)
        nc.vector.tensor_scalar_mul(g_xh, hXf, 0.5)
        g_p = gpool.tile([128, DFT * BPC], F32)
        nc.vector.tensor_scalar(g_p, g_x2, 0.044715, 1.0,
                                AluOpType.mult, AluOpType.add)
        g_u = gpool.tile([128, DFT * BPC], F32)
        nc.vector.tensor_tensor(out=g_u, in0=hXf, in1=g_p, op=AluOpType.mult)
        g_t = gpool.tile([128, DFT * BPC], F32)
        nc.scalar.activation(out=g_t, in_=g_u,
                             func=mybir.ActivationFunctionType.Tanh,
                             scale=0.7978845608028654)
        hT_sb = gpool.tile([128, DFT, BPC], F32)
        nc.vector.scalar_tensor_tensor(out=hT_sb.rearrange("p j b -> p (j b)"),
                                       in0=g_t, scalar=1.0, in1=g_xh,
                                       op0=AluOpType.add, op1=AluOpType.mult)

        # layer 2 (+ K=1 ones-row matmul folding gate_b2)
        psum_l = gps.tile([BPC, E], F32, bufs=2, tag="pst")
        for j in range(DFT):
            nc.tensor.matmul(out=psum_l, lhsT=hT_sb[:, j, :], rhs=w2_sb[:, j, :],
                             start=(j == 0), stop=False)
        nc.tensor.matmul(out=psum_l, lhsT=ones_sb[0:1, 0:BPC], rhs=b2_sb,
                         start=False, stop=True)
        logits_my = gpool.tile([BPC, E], F32)
        lev = nc.vector.tensor_copy(out=logits_my, in_=psum_l)

        # top-2 gates: g2 = sigmoid(l2 - l1), g1 = 1 - g2
        sorted8 = gpool.tile([BPC, E], F32)
        sidx = gpool.tile([BPC, E], U32)
        nc.vector.max(out=sorted8, in_=logits_my)
        nc.vector.max_index(out=sidx, in_max=sorted8, in_values=logits_my)
        diff = gpool.tile([BPC, 1], F32)
        nc.vector.tensor_tensor(out=diff, in0=sorted8[:, 1:2],
                                in1=sorted8[:, 0:1], op=AluOpType.subtract)
        g2c = gpool.tile([BPC, 1], F32)
        nc.scalar.activation(out=g2c, in_=diff,
                             func=mybir.ActivationFunctionType.Sigmoid,
                             scale=1.0)
        g1c = gpool.tile([BPC, 1], F32)
        nc.vector.tensor_scalar(g1c, g2c, -1.0, 1.0,
                                AluOpType.mult, AluOpType.add)

        # broadcast g1/g2 of each battery to all 128 partitions with one
        # matmul: rhs = [diag(g1) | diag(g2)] (8 x 16), lhsT = ones (8 x 128).
        rhs8 = gpool.tile([BPC, 2, BPC], F32)
        nc.vector.tensor_scalar_mul(rhs8[:, 0, :], id64[0:BPC, 0:BPC], g1c)
        nc.scalar.activation(out=rhs8[:, 1, :], in_=id64[0:BPC, 0:BPC],
                             func=mybir.ActivationFunctionType.Copy,
                             scale=g2c)
        psum_bc = gps.tile([128, 2 * BPC], F32, bufs=2, tag="pbc")
        nc.tensor.matmul(out=psum_bc, lhsT=ones_sb,
                         rhs=rhs8.rearrange("p s b -> p (s b)"),
                         start=True, stop=True)
        bcA = gpool.tile([128, 2, BPC], F32)
        nc.vector.tensor_copy(out=bcA.rearrange("p s b -> p (s b)"), in_=psum_bc)

        # PE warm-up gated on the logits eviction: re-ramps the HAM
        # clock-gate for the fused main-loop matmuls.
        for j in range(NWARM):
            jmm = nc.tensor.matmul(
                out=psum_h[:, 0, :], lhsT=jt[:, 0:BPC], rhs=jt,
                start=True, stop=True,
            )
            if j == 0:
                add_dep_helper(jmm.ins, lev.ins, sync=True,
                               reason="warm-up starts when logits land")

        gps_ctx.close()
        w1_ctx.close()

        # ---------- main fused phase -----------------------------------
        mps = ctx.enter_context(tc.tile_pool(name="mpsum", bufs=2, space="PSUM"))
        wbpool = ctx.enter_context(tc.tile_pool(name="wbs", bufs=2))
        scpool = ctx.enter_context(tc.tile_pool(name="scratch", bufs=2))
        opool = ctx.enter_context(tc.tile_pool(name="outs", bufs=3))

        def _vload(eng, ap, name):
            reg = eng.alloc_register(name)
            eng.reg_load(reg, ap)
            val = eng.snap(reg, donate=True)
            return nc.s_assert_within(val, 0, E - 1, skip_runtime_assert=True)

        def combine(b, pieces=2):
            """wb = g1*A_e1 + g2*A_e2 for battery b, 2 fused ops per piece."""
            rv1 = _vload(nc.vector, sidx[b:b + 1, 0:1], f"e1_{b}")
            rv2 = _vload(nc.scalar, sidx[b:b + 1, 1:2], f"e2_{b}")
            wb = wbpool.tile([128, KT, D], BF16)
            w = KT // pieces
            for h in range(pieces):
                kts = slice(h * w, (h + 1) * w)
                t2 = scpool.tile([128, w, D], BF16, tag=f"t2_{pieces}")
                nc.scalar.activation(
                    out=t2.rearrange("p k d -> p (k d)"),
                    in_=A_sb[:, ds(rv2, 1), kts, :].rearrange("p o k d -> p (o k d)"),
                    func=mybir.ActivationFunctionType.Copy,
                    scale=bcA[:, 1, b:b + 1],
                )
                nc.vector.scalar_tensor_tensor(
                    out=wb[:, kts, :].rearrange("p k d -> p (k d)"),
                    in0=A_sb[:, ds(rv1, 1), kts, :].rearrange("p o k d -> p (o k d)"),
                    scalar=bcA[:, 0, b:b + 1],
                    in1=t2.rearrange("p k d -> p (k d)"),
                    op0=AluOpType.mult, op1=AluOpType.add,
                )
            return wb

        def battery(b, wb):
            xb = xb_tiles[b]
            pm = mps.tile([128, MT, D], F32, tag="mp")
            for kt in range(KT):
                np_ = KREM if kt == 7 else 128
                for m in range(MT):
                    nc.tensor.matmul(
                        out=pm[:, m, :],
                        lhsT=xb[0:np_, kt, ts(m, 128)],
                        rhs=wb[0:np_, kt, :],
                        start=(kt == 0), stop=(kt == KT - 1),
                    )
            osb = opool.tile([128, MT, D], BF16, tag="osb")
            nc.vector.tensor_copy(
                out=osb[:, 0:2, :].rearrange("p m d -> p (m d)"),
                in_=pm[:, 0:2, :].rearrange("p m d -> p (m d)"),
            )
            nc.scalar.activation(
                out=osb[:, 2:4, :].rearrange("p m d -> p (m d)"),
                in_=pm[:, 2:4, :].rearrange("p m d -> p (m d)"),
                func=mybir.ActivationFunctionType.Copy,
            )
            return nc.sync.dma_start(
                out=outd.ap()[b].rearrange("p m d -> p (m d)"),
                in_=osb.rearrange("p m d -> p (m d)"),
            )

        wbs = {0: combine(0, pieces=4), 1: combine(1, pieces=4)}
        for b in range(BPC):
            battery(b, wbs.pop(b))
            if b + 2 < BPC:
                wbs[b + 2] = combine(b + 2)


def make_nc():
    from concourse import bacc
    nc = bacc.Bacc("TRN2", target_bir_lowering=False, debug=False,
                   num_devices=NCORES)
    build_program(nc)
    nc.finalize()
    return nc


def prep_inputs(cycle_curve_data, cycle_numbers, DKP_embeddings,
                gate_W1, gate_b1, gate_W2, gate_b2,
                expert_W, expert_b, gen_W, gen_b):
    """Host-side layout prep (reshape/pad/cast/weight-fold). Returns in_maps."""
    f32 = np.float32
    bf16 = ml_dtypes.bfloat16

    # fused expert weights A_e = gen_W + expert_W[e]; ones-row bias.
    A = np.empty((E, K, D), dtype=f32)
    A[:, :CF, :] = np.asarray(expert_W, dtype=f32) + np.asarray(gen_W, dtype=f32)
    A[:, CF, :] = np.asarray(expert_b, dtype=f32) + np.asarray(gen_b, dtype=f32)
    Abf = A.astype(bf16)
    # [128, 7(kt), E, D] so each k-tile is one contiguous DMA chunk.
    amain = np.ascontiguousarray(
        Abf[:, :896, :].reshape(E, 7, 128, D).transpose(2, 1, 0, 3))
    arem = np.ascontiguousarray(Abf[:, 896:K, :].transpose(1, 0, 2))

    # x transposed with ones-row, partition-major.
    x = np.asarray(cycle_curve_data, dtype=f32).reshape(B, L, CF)
    xT = np.empty((B, K, L), dtype=bf16)
    xT[:, :CF, :] = x.transpose(0, 2, 1).astype(bf16)
    xT[:, CF, :] = np.asarray(1.0, dtype=bf16)
    xmain = np.ascontiguousarray(
        xT[:, :896, :].reshape(B, 7, 128, L).transpose(0, 2, 1, 3))
    xrem = np.ascontiguousarray(xT[:, 896:K, :])

    # gating input, partition-major, per-core battery slice.
    g = np.zeros((GK, B), dtype=f32)
    g[:DLLM, :] = np.asarray(DKP_embeddings, dtype=f32).T
    g[DLLM, :] = np.asarray(cycle_numbers, dtype=f32)[:, 0]
    g[DLLM + 1, :] = 1.0
    gpm = g.reshape(GKT, 128, B).transpose(1, 0, 2).astype(np.float16)

    # full W1 (replicated to every core), partition-major.
    W1p = np.zeros((GK, DFF), dtype=f32)
    W1p[:DLLM + 1, :] = np.asarray(gate_W1, dtype=f32)
    W1p[DLLM + 1, :] = np.asarray(gate_b1, dtype=f32)
    w1f = np.ascontiguousarray(
        W1p.reshape(GKT, 128, DFF).transpose(1, 0, 2)
        .reshape(128, GKT * DFF).astype(np.float16))

    w2f = np.ascontiguousarray(
        np.asarray(gate_W2, dtype=f32).reshape(DFT, 128, E)
        .transpose(1, 0, 2).reshape(128, DFT * E))
    b2row = np.ascontiguousarray(np.asarray(gate_b2, dtype=f32).reshape(1, E))
    id64 = np.eye(B, dtype=f32)

    in_maps = []
    for c in range(NCORES):
        ginp = np.ascontiguousarray(
            gpm[:, :, c * BPC:(c + 1) * BPC].reshape(128, GKT * BPC))
        in_maps.append({
            "xmain": np.ascontiguousarray(xmain[c * BPC:(c + 1) * BPC]),
            "xrem": np.ascontiguousarray(xrem[c * BPC:(c + 1) * BPC]),
            "amain": amain,
            "arem": arem,
            "ginp": ginp,
            "w1f": w1f,
            "w2f": w2f,
            "b2row": b2row,
            "id64": id64,
        })
    return in_maps


_CACHED = {}


def run(inputs, trace=False, tmpdir=None):
    """Run on the 8 NeuronCores; returns (full_output, BassKernelResults)."""
    from concourse import bass_utils
    in_maps = prep_inputs(**inputs)
    nc = _CACHED.get("nc")
    if nc is None:
        nc = make_nc()
        _CACHED["nc"] = nc
    res = bass_utils.run_bass_kernel_spmd(
        nc, in_maps, core_ids=list(range(NCORES)), trace=trace, tmpdir=tmpdir
    )
    outs = [np.asarray(r["out"]) for r in res.results]
    full = np.concatenate(outs, axis=0)          # [B, 128, MT, D] bf16
    full = full.transpose(0, 2, 1, 3).reshape(B, L, D).astype(np.float32)
    return full, res


def kernel(**inputs):
    full, _ = run(inputs, trace=False)
    return full
